# revision 29
# baseline (speedup 1.0000x reference)
"""EdgeConv GNN (4 layers) on 8 Trainium2 NeuronCores.

Algebraic restructure: with y = x @ theta_w.T and
v = x @ (phi_w - theta_w).T + (phi_b + theta_b),
    msg_e = theta(x[src]-x[dst]) + theta_b + phi(x[dst]) + phi_b
          = y[src] + v[dst]
and since v[dst] is constant within a dst segment:
    out = relu(v + segment_max(y[src], dst))
(nodes with no in-edges come out of segment_max at -1e30 -> relu -> 0,
matching the reference's where(isneginf, 0) + relu).

Distribution: nodes sharded by dst across 8 cores (graph parallel).
Each layer: per-core matmuls produce the y-shard in bf16 -> AllGather
the full y table to every core's DRAM -> SWDGE dma_gather of y rows by
src in dst-sorted slot order -> strided reduce_max per 128-node block.

Perf structure:
  - dma_gather desc-gen runs on Q7 cpu pairs selected by queue_num;
    num_swdge_queues=4 + round-robin queue_num parallelizes desc-gen 4x.
  - y table is bf16: halves gather DMA bytes, AllGather, and DVE reduce.
  - x is kept only transposed (xT); the per-block PE transpose runs in
    the reduce phase (PE idle there), so the matmul phase is just
    49 back-to-back matmuls + y writes.

dma_gather indices are int16 (<= 32767) so the 50176-row table is
addressed through two windows: A = rows [0, 32768) (src cores 0-4) and
B = rows [18816, 50176) (src cores 3-7); flex edges (src cores 3-4)
balance the windows per dst. Node order per core: phantoms at positions
0..21, real nodes sorted by (dA, snake(dB)) so both windows' per-block
max degrees stay tight. Per-core slot structure must be identical
across cores (single SPMD instruction stream), so block degree caps are
maxima across all 8 cores.
"""

import numpy as np

N = 50000
NCORES = 8
NPC = 6250            # real nodes per core
NPCP = 6272           # padded nodes per core (49 * 128)
F = 128
NL = 4
NB = NPCP // 128      # 49 blocks per core
NTAB = NCORES * NPCP  # 50176 table rows
BASE_B = 3 * NPCP     # 18816: window B base row
N_PHANTOM = NPCP - NPC
CAP = 24              # max chunks per gather tile (per window, per group)
NQ = 4                # SWDGE queues (Q7 cpu pairs)
NEG = -1.0e30

_cache = {}


# ----------------------------------------------------------------------------
# host-side graph preprocessing
# ----------------------------------------------------------------------------

def _prep_graph(src, dst):
    src = np.asarray(src).astype(np.int64)
    dst = np.asarray(dst).astype(np.int64)
    s_core = src // NPC
    d_core = dst // NPC

    fixedA = s_core <= 2
    flex = (s_core == 3) | (s_core == 4)
    dA0 = np.bincount(dst[fixedA], minlength=N)
    dB0 = np.bincount(dst[s_core >= 5], minlength=N)
    dfx = np.bincount(dst[flex], minlength=N)
    kAf = np.clip((dB0 - dA0 + dfx + 1) // 2, 0, dfx)
    dA = dA0 + kAf
    dB = dB0 + (dfx - kAf)

    # edge side: fixed by src core; flex edges ranked within their dst group
    sideA = fixedA.copy()
    fe = np.flatnonzero(flex)
    fe = fe[np.argsort(dst[fe], kind="stable")]
    dsf = dst[fe]
    starts = np.r_[0, np.flatnonzero(np.diff(dsf)) + 1]
    runlen = np.diff(np.r_[starts, len(dsf)])
    rank = np.arange(len(dsf)) - np.repeat(starts, runlen)
    sideA[fe[rank < kAf[dsf]]] = True

    # per-core node order: phantoms at positions 0..21, real nodes sorted by
    # (dA desc, snake(dB)): dB alternates asc/desc per dA run to smooth block
    # maxima; descending so the last blocks (end-of-layer tail) are light
    pos = np.empty(N, np.int64)
    for c in range(NCORES):
        ids = np.arange(c * NPC, (c + 1) * NPC)
        a, b = dA[ids], dB[ids]
        order = np.lexsort((np.where(a % 2 == 0, b, -b), -a))
        pos[ids[order]] = N_PHANTOM + np.arange(NPC)
    sig = (np.arange(N) // NPC) * NPCP + pos  # orig node -> table row
    blk = pos // 128
    lane = pos % 128

    # global (cross-core) block degree caps
    KA = np.zeros(NB, np.int64)
    KB = np.zeros(NB, np.int64)
    np.maximum.at(KA, blk, dA)
    np.maximum.at(KB, blk, dB)
    cbA = np.r_[0, np.cumsum(KA)]
    cbB = np.r_[0, np.cumsum(KB)]
    CA, CB = int(cbA[-1]), int(cbB[-1])
    assert KA.max() <= CAP and KB.max() <= CAP, (KA.max(), KB.max())

    # slot arrays (per core); dummy rows are phantom rows (-1e30):
    #   window A dummy: table row 0;  window B dummy: row 4*NPCP - BASE_B
    idxA = np.zeros((NCORES, CA * 128), np.int16)
    idxB = np.full((NCORES, CB * 128), (4 * NPCP) - BASE_B, np.int16)

    for side, idx_arr, cb, base in ((True, idxA, cbA, 0), (False, idxB, cbB, BASE_B)):
        e = np.flatnonzero(sideA == side)
        e = e[np.argsort(dst[e], kind="stable")]
        de = dst[e]
        starts = np.r_[0, np.flatnonzero(np.diff(de)) + 1]
        runlen = np.diff(np.r_[starts, len(de)])
        rank = np.arange(len(de)) - np.repeat(starts, runlen)
        slot = (cb[blk[de]] + rank) * 128 + lane[de]
        val = sig[src[e]] - base
        assert val.min() >= 0 and val.max() < 32768, (val.min(), val.max())
        idx_arr[d_core[e], slot] = val.astype(np.int16)

    # wrap indices: [n] -> [128, n//16] int16, replicated across 8 groups of 16
    def wrap(a):
        n = a.shape[1]
        w = a.reshape(NCORES, n // 16, 16).transpose(0, 2, 1)  # [c, 16, n/16]
        return np.ascontiguousarray(
            np.broadcast_to(w[:, None, :, :], (NCORES, 8, 16, n // 16))
        ).reshape(NCORES, 128, n // 16)

    # gather groups: consecutive blocks, chunk budget CAP per window; the
    # final group is kept to <= 2 blocks so the end-of-layer tail is short
    groups = []
    b0 = 0
    while b0 < NB:
        lim = NB - 2 if b0 < NB - 2 else NB
        nb = 1
        while (
            b0 + nb < lim
            and cbA[b0 + nb + 1] - cbA[b0] <= CAP
            and cbB[b0 + nb + 1] - cbB[b0] <= CAP
        ):
            nb += 1
        groups.append((b0, nb, int(cbA[b0]), int(cbA[b0 + nb] - cbA[b0]),
                       int(cbB[b0]), int(cbB[b0 + nb] - cbB[b0])))
        b0 += nb

    return dict(
        sig=sig, pos=pos, KA=KA, KB=KB, cbA=cbA, cbB=cbB, CA=CA, CB=CB,
        idxA=wrap(idxA), idxB=wrap(idxB), groups=groups,
        idxA_flat=idxA, idxB_flat=idxB,
    )


def _prep_weights(theta_w, theta_b, phi_w, phi_b):
    theta_w = np.asarray(theta_w, np.float32)
    phi_w = np.asarray(phi_w, np.float32)
    cb = (np.asarray(theta_b, np.float32) + np.asarray(phi_b, np.float32))
    wcat = np.concatenate(
        [theta_w.transpose(0, 2, 1), (phi_w - theta_w).transpose(0, 2, 1)], axis=2
    )  # [NL, 128(in), 256(out: y|v)]
    return np.ascontiguousarray(wcat), np.ascontiguousarray(cb)


# ----------------------------------------------------------------------------
# device kernel
# ----------------------------------------------------------------------------

def _build_kernel(g, repeats=1):
    import concourse.bacc as bacc
    import concourse.mybir as mybir
    import concourse.tile as tile
    from concourse.masks import make_identity

    KA, KB, groups = g["KA"], g["KB"], g["groups"]
    CA, CB = g["CA"], g["CB"]

    nc = bacc.Bacc("TRN2", target_bir_lowering=False, debug=False,
                   num_devices=NCORES, num_swdge_queues=NQ)

    xin = nc.dram_tensor("xin", [NPCP, F], mybir.dt.float32, kind="ExternalInput")
    idxA_in = nc.dram_tensor("idxA", [128, CA * 8], mybir.dt.int16, kind="ExternalInput")
    idxB_in = nc.dram_tensor("idxB", [128, CB * 8], mybir.dt.int16, kind="ExternalInput")
    wcat_in = nc.dram_tensor("wcat", [NL, F, 2 * F], mybir.dt.float32, kind="ExternalInput")
    cb_in = nc.dram_tensor("cb", [NL, F], mybir.dt.float32, kind="ExternalInput")
    xout = nc.dram_tensor("xout", [NPCP, F], mybir.dt.float32, kind="ExternalOutput")
    xout_r = xout.rearrange("(b p) f -> p b f", p=128)

    fp32 = mybir.dt.float32
    bf16 = mybir.dt.bfloat16
    Alu = mybir.AluOpType
    Act = mybir.ActivationFunctionType
    qctr = [0]

    with tile.TileContext(nc) as tc:
        with (
            tc.tile_pool(name="const", bufs=1) as constp,
            tc.tile_pool(name="xTp", bufs=2) as xTp,
            tc.tile_pool(name="vp", bufs=2) as vp,
            tc.tile_pool(name="wp", bufs=2) as wp,
            tc.tile_pool(name="yp", bufs=3) as yp,
            tc.tile_pool(name="ga", bufs=4) as gap,
            tc.tile_pool(name="gb", bufs=4) as gbp,
            tc.tile_pool(name="tp", bufs=6) as tp,
            tc.tile_pool(name="pp", bufs=4) as pp,
            tc.tile_pool(name="ps", bufs=4, space="PSUM") as ps,
            tc.tile_pool(name="dram", bufs=2, space="DRAM") as dram,
        ):
            ident = constp.tile([128, 128], fp32)
            make_identity(nc, ident[:])
            idxA = constp.tile([128, CA * 8], mybir.dt.int16)
            idxB = constp.tile([128, CB * 8], mybir.dt.int16)
            nc.sync.dma_start(idxA[:], idxA_in[:])
            nc.sync.dma_start(idxB[:], idxB_in[:])

            # all layers' weights upfront
            Ws, cbbcs = [], []
            for l in range(NL):
                W = constp.tile([128, 2 * F], fp32, tag=f"w{l}")
                nc.sync.dma_start(W[:], wcat_in[l])
                cb_sb = wp.tile([1, F], fp32, tag="cb")
                nc.sync.dma_start(cb_sb[:], cb_in[l : l + 1, :])
                cbbc = constp.tile([128, F], fp32, tag=f"cbbc{l}")
                nc.gpsimd.partition_broadcast(cbbc[:], cb_sb[:])
                Ws.append(W)
                cbbcs.append(cbbc)

            def window_max(gT, k0, h, tag):
                """max over h chunks of gT[:, k0:k0+h, :] -> [128, F] bf16 AP.

                h >= 6 first does a unit-stride pair-max pass (reads 256B
                contiguous runs) to halve the slow strided tensor_reduce.
                """
                if h == 1:
                    return gT[:, k0, :]
                t = tp.tile([128, F], bf16, tag=tag)
                if h >= 6:
                    m = h // 2
                    pr = pp.tile([128, 8, F], bf16, tag="p" + tag)
                    pair = gT[:, k0 : k0 + 2 * m, :].rearrange(
                        "p (c two) f -> p c (two f)", two=2)
                    nc.vector.tensor_tensor(out=pr[:, 0:m, :], in0=pair[:, :, 0:F],
                                            in1=pair[:, :, F : 2 * F], op=Alu.max)
                    nc.vector.tensor_reduce(
                        out=t[:], in_=pr[:, 0:m, :].rearrange("p c f -> p f c"),
                        axis=mybir.AxisListType.X, op=Alu.max)
                    if h % 2:
                        nc.vector.tensor_tensor(out=t[:], in0=t[:],
                                                in1=gT[:, k0 + 2 * m, :], op=Alu.max)
                else:
                    nc.vector.tensor_reduce(
                        out=t[:], in_=gT[:, k0 : k0 + h, :].rearrange("p c f -> p f c"),
                        axis=mybir.AxisListType.X, op=Alu.max)
                return t[:]

            def mm_block(t, xT, W, cbbc, v, y_ag_in):
                """y/v production for one 128-node block (layer matmul)."""
                yv_ps = ps.tile([128, 2 * F], fp32, tag="yv_ps")
                nc.tensor.matmul(yv_ps[:], lhsT=xT[:, t, :], rhs=W[:],
                                 start=True, stop=True)
                y_sb = yp.tile([128, F], bf16, tag="y")
                nc.scalar.activation(y_sb[:], yv_ps[:, 0:F], Act.Copy)
                if t == 0:
                    nc.vector.memset(y_sb[0:N_PHANTOM, :], NEG)
                nc.sync.dma_start(y_ag_in[t * 128 : (t + 1) * 128, :], y_sb[:])
                nc.vector.tensor_tensor(out=v[:, t, :], in0=yv_ps[:, F : 2 * F],
                                        in1=cbbc[:], op=Alu.add)

            # ---- layer-0 prologue: transpose x0, produce y0/v0 ----
            # x0 borrows a v-pool buffer: dead after the prologue transposes,
            # and the pool's bufs=2 rotation only reuses it at layer 1's vn.
            x0 = vp.tile([128, NB, F], fp32, tag="v")
            nc.sync.dma_start(x0[:], xin.rearrange("(b p) f -> p b f", p=128))
            xT = xTp.tile([128, NB, 128], fp32, tag="xT")
            y_ag_in = dram.tile([NPCP, F], bf16, tag="yag")
            v = vp.tile([128, NB, F], fp32, tag="v")
            # transposes first, then matmuls: batching per engine avoids the
            # per-block PE<->Scalar convoy (xT holds all 49 blocks)
            for t in range(NB):
                xT_ps = ps.tile([128, 128], fp32, tag="xt_ps")
                nc.tensor.transpose(xT_ps[:], x0[:, t, :], ident[:])
                nc.scalar.activation(xT[:, t, :], xT_ps[:], Act.Copy)
            for t in range(NB):
                mm_block(t, xT, Ws[0], cbbcs[0], v, y_ag_in)

            for lt in range(NL * repeats):
                l = lt % NL
                last = lt == NL * repeats - 1
                nl = (lt + 1) % NL

                y_all = dram.tile([NTAB, F], bf16, tag="yall", addr_space="Shared")
                nc.gpsimd.collective_compute(
                    "AllGather",
                    Alu.bypass,
                    replica_groups=[list(range(NCORES))],
                    ins=[y_ag_in.opt()],
                    outs=[y_all.opt()],
                )

                # ---- gather + segment-max (+ next-layer matmul) phase ----
                if not last:
                    xT = xTp.tile([128, NB, 128], fp32, tag="xT")
                    y_ag_in = dram.tile([NPCP, F], bf16, tag="yag")
                    vn = vp.tile([128, NB, F], fp32, tag="v")
                for (b0, nbl, aoff, acnt, boff, bcnt) in groups:
                    gA = gap.tile([128, CAP, F], bf16, tag="ga")
                    gB = gbp.tile([128, CAP, F], bf16, tag="gb")
                    # Q7 gather ucode scratch caps num_idxs at 1024 (8 chunks)
                    for o in range(0, acnt, 8):
                        n = min(8, acnt - o)
                        nc.gpsimd.dma_gather(
                            gA[:, o : o + n, :], y_all[:, :],
                            idxA[:, (aoff + o) * 8 : (aoff + o + n) * 8],
                            n * 128, n * 128, F,
                            queue_num=qctr[0] % NQ,
                        )
                        qctr[0] += 1
                    for o in range(0, bcnt, 8):
                        n = min(8, bcnt - o)
                        nc.gpsimd.dma_gather(
                            gB[:, o : o + n, :], y_all[BASE_B:, :],
                            idxB[:, (boff + o) * 8 : (boff + o + n) * 8],
                            n * 128, n * 128, F,
                            queue_num=qctr[0] % NQ,
                        )
                        qctr[0] += 1
                    ka = 0
                    kb = 0
                    wmax = window_max
                    for b in range(b0, b0 + nbl):
                        ha, hb = int(KA[b]), int(KB[b])
                        tS = tp.tile([128, F], fp32, tag="ts")
                        if ha > 0 and hb > 0:
                            rA = wmax(gA, ka, ha, "ta")
                            rB = wmax(gB, kb, hb, "tb")
                            tM = tp.tile([128, F], fp32, tag="tm")
                            nc.vector.tensor_tensor(out=tM[:], in0=rA, in1=rB,
                                                    op=Alu.max)
                            nc.vector.tensor_tensor(out=tS[:], in0=tM[:],
                                                    in1=v[:, b, :], op=Alu.add)
                        elif ha > 0 or hb > 0:
                            rA = (wmax(gA, ka, ha, "ta") if ha > 0
                                  else wmax(gB, kb, hb, "tb"))
                            nc.vector.tensor_tensor(out=tS[:], in0=rA,
                                                    in1=v[:, b, :], op=Alu.add)
                        else:
                            nc.vector.memset(tS[:], NEG)
                        if last:
                            xn = tp.tile([128, F], fp32, tag="xn")
                            nc.scalar.activation(xn[:], tS[:], Act.Relu)
                            nc.sync.dma_start(xout_r[:, b, :], xn[:])
                        else:
                            xT_ps = ps.tile([128, 128], fp32, tag="xt_ps")
                            nc.tensor.transpose(xT_ps[:], tS[:], ident[:])
                            nc.scalar.activation(xT[:, b, :], xT_ps[:], Act.Relu)
                            mm_block(b, xT, Ws[nl], cbbcs[nl], vn, y_ag_in)
                        ka += ha
                        kb += hb
                if not last:
                    v = vn

    nc.compile()
    return nc


# ----------------------------------------------------------------------------
# numpy emulation of the device dataflow (for validating prep structures)
# ----------------------------------------------------------------------------

def _bf16(x):
    u = x.astype(np.float32).view(np.uint32)
    u = (u + 0x8000 + ((u >> 16) & 1)) & 0xFFFF0000
    return u.view(np.float32)


def _emulate(g, feats_dev, wcat, cb):
    KA, KB = g["KA"], g["KB"]
    x = feats_dev.copy()  # [NCORES, NPCP, F] sigma-ordered
    for l in range(NL):
        y_sh = _bf16(np.einsum("cnf,fk->cnk", x, wcat[l, :, :F]))
        v = np.einsum("cnf,fk->cnk", x, wcat[l, :, F:]) + cb[l]
        y_sh[:, :N_PHANTOM, :] = NEG
        table = y_sh.reshape(NTAB, F)
        xn = np.empty_like(x)
        for c in range(NCORES):
            gA = table[g["idxA_flat"][c].astype(np.int64)]
            gB = table[BASE_B + g["idxB_flat"][c].astype(np.int64)]
            gA = gA.reshape(g["CA"], 128, F)
            gB = gB.reshape(g["CB"], 128, F)
            for b in range(NB):
                a0, b0 = g["cbA"][b], g["cbB"][b]
                parts = []
                if KA[b] > 0:
                    parts.append(gA[a0 : a0 + KA[b]].max(0))
                if KB[b] > 0:
                    parts.append(gB[b0 : b0 + KB[b]].max(0))
                agg = np.full((128, F), NEG, np.float32) if not parts else (
                    parts[0] if len(parts) == 1 else np.maximum(*parts))
                xn[c, b * 128 : (b + 1) * 128] = np.maximum(
                    agg + v[c, b * 128 : (b + 1) * 128], 0.0)
        x = xn
    return x


def _make_in_maps(g, feats_dev, wcat, cb):
    in_maps = []
    for c in range(NCORES):
        in_maps.append({
            "xin": np.ascontiguousarray(feats_dev[c]),
            "idxA": np.ascontiguousarray(g["idxA"][c]),
            "idxB": np.ascontiguousarray(g["idxB"][c]),
            "wcat": wcat,
            "cb": cb,
        })
    return in_maps


def _feats_dev(g, feats):
    feats = np.asarray(feats, np.float32)
    fd = np.zeros((NCORES, NPCP, F), np.float32)
    core = np.arange(N) // NPC
    fd[core, g["pos"]] = feats
    return fd


def _assemble(g, results):
    out_sh = np.stack([r["xout"] for r in results])  # [NCORES, NPCP, F]
    core = np.arange(N) // NPC
    return np.ascontiguousarray(out_sh[core, g["pos"]])


def run(feats, src, dst, theta_w, theta_b, phi_w, phi_b, trace=False, repeats=1):
    from concourse.bass_utils import run_bass_kernel_spmd

    src = np.asarray(src)
    dst = np.asarray(dst)
    key = (src.tobytes()[:64], dst.tobytes()[:64], len(src))
    if _cache.get("graph_key") != key:
        _cache.clear()
        _cache["graph"] = _prep_graph(src, dst)
        _cache["graph_key"] = key
    g = _cache["graph"]
    nck = ("nc", repeats)
    if nck not in _cache:
        _cache[nck] = _build_kernel(g, repeats=repeats)
    nc = _cache[nck]

    wcat, cb = _prep_weights(theta_w, theta_b, phi_w, phi_b)
    feats_dev = _feats_dev(g, feats)
    in_maps = _make_in_maps(g, feats_dev, wcat, cb)
    res = run_bass_kernel_spmd(nc, in_maps, core_ids=list(range(NCORES)),
                               trace=trace)
    out = _assemble(g, res.results)
    return out, res


def kernel(feats, src, dst, theta_w, theta_b, phi_w, phi_b):
    out, _ = run(feats, src, dst, theta_w, theta_b, phi_w, phi_b)
    return out


# revision 30
# speedup vs baseline: 1.2305x; 1.2305x over previous
"""EdgeConv GNN (4 layers) on 8 Trainium2 NeuronCores.

Algebraic restructure: with y = x @ theta_w.T and
v = x @ (phi_w - theta_w).T + (phi_b + theta_b),
    msg_e = theta(x[src]-x[dst]) + theta_b + phi(x[dst]) + phi_b
          = y[src] + v[dst]
and since v[dst] is constant within a dst segment:
    out = relu(v + segment_max(y[src], dst))
(nodes with no in-edges come out of segment_max at -1e30 -> relu -> 0,
matching the reference's where(isneginf, 0) + relu).

Distribution: nodes sharded by dst across 8 cores (graph parallel).
Each layer: per-core matmuls produce the y-shard in bf16 -> AllGather
the full y table to every core's DRAM -> SWDGE dma_gather of y rows by
src in dst-sorted slot order -> strided reduce_max per 128-node block.

Perf structure:
  - dma_gather desc-gen runs on Q7 cpu pairs selected by queue_num;
    num_swdge_queues=4 + round-robin queue_num parallelizes desc-gen 4x.
  - y table is bf16: halves gather DMA bytes, AllGather, and DVE reduce.
  - x is kept only transposed (xT); the per-block PE transpose runs in
    the reduce phase (PE idle there), so the matmul phase is just
    49 back-to-back matmuls + y writes.

dma_gather indices are int16 (<= 32767) so the 50176-row table is
addressed through two windows: A = rows [0, 32768) (src cores 0-4) and
B = rows [18816, 50176) (src cores 3-7); flex edges (src cores 3-4)
balance the windows per dst. Node order per core: phantoms at positions
0..21, real nodes sorted by (dA, snake(dB)) so both windows' per-block
max degrees stay tight. Per-core slot structure must be identical
across cores (single SPMD instruction stream), so block degree caps are
maxima across all 8 cores.
"""

import numpy as np

N = 50000
NCORES = 8
NPC = 6250            # real nodes per core
NPCP = 6272           # padded nodes per core (49 * 128)
F = 128
NL = 4
NB = NPCP // 128      # 49 blocks per core
NTAB = NCORES * NPCP  # 50176 table rows
BASE_B = 3 * NPCP     # 18816: window B base row
N_PHANTOM = NPCP - NPC
CAP = 24              # max chunks per gather tile (per window, per group)
NQ = 4                # SWDGE queues (Q7 cpu pairs)
NEG = -1.0e30

_cache = {}


# ----------------------------------------------------------------------------
# host-side graph preprocessing
# ----------------------------------------------------------------------------

def _prep_graph(src, dst):
    src = np.asarray(src).astype(np.int64)
    dst = np.asarray(dst).astype(np.int64)
    s_core = src // NPC
    d_core = dst // NPC

    fixedA = s_core <= 2
    flex = (s_core == 3) | (s_core == 4)
    dA0 = np.bincount(dst[fixedA], minlength=N)
    dB0 = np.bincount(dst[s_core >= 5], minlength=N)
    dfx = np.bincount(dst[flex], minlength=N)
    kAf = np.clip((dB0 - dA0 + dfx + 1) // 2, 0, dfx)
    dA = dA0 + kAf
    dB = dB0 + (dfx - kAf)

    # edge side: fixed by src core; flex edges ranked within their dst group
    sideA = fixedA.copy()
    fe = np.flatnonzero(flex)
    fe = fe[np.argsort(dst[fe], kind="stable")]
    dsf = dst[fe]
    starts = np.r_[0, np.flatnonzero(np.diff(dsf)) + 1]
    runlen = np.diff(np.r_[starts, len(dsf)])
    rank = np.arange(len(dsf)) - np.repeat(starts, runlen)
    sideA[fe[rank < kAf[dsf]]] = True

    # per-core node order: phantoms at positions 0..21, real nodes sorted by
    # (max(dA,dB) desc, snake(min)): the secondary key alternates asc/desc per
    # primary run to smooth block maxima; descending primary keeps the last
    # blocks (end-of-layer tail) light in BOTH windows
    pos = np.empty(N, np.int64)
    for c in range(NCORES):
        ids = np.arange(c * NPC, (c + 1) * NPC)
        a, b = dA[ids], dB[ids]
        hi, lo = np.maximum(a, b), np.minimum(a, b)
        order = np.lexsort((np.where(hi % 2 == 0, lo, -lo), -hi))
        pos[ids[order]] = N_PHANTOM + np.arange(NPC)
    sig = (np.arange(N) // NPC) * NPCP + pos  # orig node -> table row
    blk = pos // 128
    lane = pos % 128

    # global (cross-core) block degree caps
    KA = np.zeros(NB, np.int64)
    KB = np.zeros(NB, np.int64)
    np.maximum.at(KA, blk, dA)
    np.maximum.at(KB, blk, dB)
    cbA = np.r_[0, np.cumsum(KA)]
    cbB = np.r_[0, np.cumsum(KB)]
    CA, CB = int(cbA[-1]), int(cbB[-1])
    assert KA.max() <= CAP and KB.max() <= CAP, (KA.max(), KB.max())

    # slot arrays (per core); dummy rows are phantom rows (-1e30):
    #   window A dummy: table row 0;  window B dummy: row 4*NPCP - BASE_B
    idxA = np.zeros((NCORES, CA * 128), np.int16)
    idxB = np.full((NCORES, CB * 128), (4 * NPCP) - BASE_B, np.int16)

    for side, idx_arr, cb, base in ((True, idxA, cbA, 0), (False, idxB, cbB, BASE_B)):
        e = np.flatnonzero(sideA == side)
        e = e[np.argsort(dst[e], kind="stable")]
        de = dst[e]
        starts = np.r_[0, np.flatnonzero(np.diff(de)) + 1]
        runlen = np.diff(np.r_[starts, len(de)])
        rank = np.arange(len(de)) - np.repeat(starts, runlen)
        slot = (cb[blk[de]] + rank) * 128 + lane[de]
        val = sig[src[e]] - base
        assert val.min() >= 0 and val.max() < 32768, (val.min(), val.max())
        idx_arr[d_core[e], slot] = val.astype(np.int16)

    # wrap indices: [n] -> [128, n//16] int16, replicated across 8 groups of 16
    def wrap(a):
        n = a.shape[1]
        w = a.reshape(NCORES, n // 16, 16).transpose(0, 2, 1)  # [c, 16, n/16]
        return np.ascontiguousarray(
            np.broadcast_to(w[:, None, :, :], (NCORES, 8, 16, n // 16))
        ).reshape(NCORES, 128, n // 16)

    # gather groups: consecutive blocks, chunk budget CAP per window; the
    # final group is kept to <= 2 blocks so the end-of-layer tail is short
    groups = []
    b0 = 0
    while b0 < NB:
        lim = NB - 2 if b0 < NB - 2 else NB
        nb = 1
        while (
            b0 + nb < lim
            and cbA[b0 + nb + 1] - cbA[b0] <= CAP
            and cbB[b0 + nb + 1] - cbB[b0] <= CAP
        ):
            nb += 1
        groups.append((b0, nb, int(cbA[b0]), int(cbA[b0 + nb] - cbA[b0]),
                       int(cbB[b0]), int(cbB[b0 + nb] - cbB[b0])))
        b0 += nb

    return dict(
        sig=sig, pos=pos, KA=KA, KB=KB, cbA=cbA, cbB=cbB, CA=CA, CB=CB,
        idxA=wrap(idxA), idxB=wrap(idxB), groups=groups,
        idxA_flat=idxA, idxB_flat=idxB,
    )


def _prep_weights(theta_w, theta_b, phi_w, phi_b):
    theta_w = np.asarray(theta_w, np.float32)
    phi_w = np.asarray(phi_w, np.float32)
    cb = (np.asarray(theta_b, np.float32) + np.asarray(phi_b, np.float32))
    wcat = np.concatenate(
        [theta_w.transpose(0, 2, 1), (phi_w - theta_w).transpose(0, 2, 1)], axis=2
    )  # [NL, 128(in), 256(out: y|v)]
    return np.ascontiguousarray(wcat), np.ascontiguousarray(cb)


# ----------------------------------------------------------------------------
# device kernel
# ----------------------------------------------------------------------------

def _build_kernel(g, repeats=1):
    import concourse.bacc as bacc
    import concourse.mybir as mybir
    import concourse.tile as tile
    from concourse.masks import make_identity

    KA, KB, groups = g["KA"], g["KB"], g["groups"]
    CA, CB = g["CA"], g["CB"]

    nc = bacc.Bacc("TRN2", target_bir_lowering=False, debug=False,
                   num_devices=NCORES, num_swdge_queues=NQ)

    xin = nc.dram_tensor("xin", [NPCP, F], mybir.dt.float32, kind="ExternalInput")
    idxA_in = nc.dram_tensor("idxA", [128, CA * 8], mybir.dt.int16, kind="ExternalInput")
    idxB_in = nc.dram_tensor("idxB", [128, CB * 8], mybir.dt.int16, kind="ExternalInput")
    wcat_in = nc.dram_tensor("wcat", [NL, F, 2 * F], mybir.dt.float32, kind="ExternalInput")
    cb_in = nc.dram_tensor("cb", [NL, F], mybir.dt.float32, kind="ExternalInput")
    xout = nc.dram_tensor("xout", [NPCP, F], mybir.dt.float32, kind="ExternalOutput")
    xout_r = xout.rearrange("(b p) f -> p b f", p=128)

    fp32 = mybir.dt.float32
    bf16 = mybir.dt.bfloat16
    Alu = mybir.AluOpType
    Act = mybir.ActivationFunctionType
    qctr = [0]

    with tile.TileContext(nc) as tc:
        with (
            tc.tile_pool(name="const", bufs=1) as constp,
            tc.tile_pool(name="xTp", bufs=2) as xTp,
            tc.tile_pool(name="vp", bufs=2) as vp,
            tc.tile_pool(name="wp", bufs=2) as wp,
            tc.tile_pool(name="yp", bufs=3) as yp,
            tc.tile_pool(name="ga", bufs=4) as gap,
            tc.tile_pool(name="gb", bufs=4) as gbp,
            tc.tile_pool(name="tp", bufs=6) as tp,
            tc.tile_pool(name="pp", bufs=4) as pp,
            tc.tile_pool(name="ps", bufs=4, space="PSUM") as ps,
            tc.tile_pool(name="dram", bufs=2, space="DRAM") as dram,
        ):
            ident = constp.tile([128, 128], fp32)
            make_identity(nc, ident[:])
            idxA = constp.tile([128, CA * 8], mybir.dt.int16)
            idxB = constp.tile([128, CB * 8], mybir.dt.int16)
            nc.sync.dma_start(idxA[:], idxA_in[:])
            nc.sync.dma_start(idxB[:], idxB_in[:])

            # all layers' weights upfront
            Ws, cbbcs = [], []
            for l in range(NL):
                W = constp.tile([128, 2 * F], fp32, tag=f"w{l}")
                nc.sync.dma_start(W[:], wcat_in[l])
                cb_sb = wp.tile([1, F], fp32, tag="cb")
                nc.sync.dma_start(cb_sb[:], cb_in[l : l + 1, :])
                cbbc = constp.tile([128, F], fp32, tag=f"cbbc{l}")
                nc.gpsimd.partition_broadcast(cbbc[:], cb_sb[:])
                Ws.append(W)
                cbbcs.append(cbbc)

            def window_max(gT, k0, h, tag):
                """max over h chunks of gT[:, k0:k0+h, :] -> [128, F] bf16 AP.

                h >= 6 first does a unit-stride pair-max pass (reads 256B
                contiguous runs) to halve the slow strided tensor_reduce.
                """
                if h == 1:
                    return gT[:, k0, :]
                t = tp.tile([128, F], bf16, tag=tag)
                if h >= 6:
                    m = h // 2
                    pr = pp.tile([128, 8, F], bf16, tag="p" + tag)
                    pair = gT[:, k0 : k0 + 2 * m, :].rearrange(
                        "p (c two) f -> p c (two f)", two=2)
                    nc.vector.tensor_tensor(out=pr[:, 0:m, :], in0=pair[:, :, 0:F],
                                            in1=pair[:, :, F : 2 * F], op=Alu.max)
                    nc.vector.tensor_reduce(
                        out=t[:], in_=pr[:, 0:m, :].rearrange("p c f -> p f c"),
                        axis=mybir.AxisListType.X, op=Alu.max)
                    if h % 2:
                        nc.vector.tensor_tensor(out=t[:], in0=t[:],
                                                in1=gT[:, k0 + 2 * m, :], op=Alu.max)
                else:
                    nc.vector.tensor_reduce(
                        out=t[:], in_=gT[:, k0 : k0 + h, :].rearrange("p c f -> p f c"),
                        axis=mybir.AxisListType.X, op=Alu.max)
                return t[:]

            def mm_block(t, xT, W, cbbc, v, y_ag_in):
                """y/v production for one 128-node block (layer matmul)."""
                yv_ps = ps.tile([128, 2 * F], fp32, tag="yv_ps")
                nc.tensor.matmul(yv_ps[:], lhsT=xT[:, t, :], rhs=W[:],
                                 start=True, stop=True)
                y_sb = yp.tile([128, F], bf16, tag="y")
                nc.scalar.activation(y_sb[:], yv_ps[:, 0:F], Act.Copy)
                if t == 0:
                    nc.vector.memset(y_sb[0:N_PHANTOM, :], NEG)
                nc.sync.dma_start(y_ag_in[t * 128 : (t + 1) * 128, :], y_sb[:])
                nc.vector.tensor_tensor(out=v[:, t, :], in0=yv_ps[:, F : 2 * F],
                                        in1=cbbc[:], op=Alu.add)

            # ---- layer-0 prologue: transpose x0, produce y0/v0 ----
            # x0 borrows a v-pool buffer: dead after the prologue transposes,
            # and the pool's bufs=2 rotation only reuses it at layer 1's vn.
            x0 = vp.tile([128, NB, F], fp32, tag="v")
            nc.sync.dma_start(x0[:], xin.rearrange("(b p) f -> p b f", p=128))
            xT = xTp.tile([128, NB, 128], fp32, tag="xT")
            y_ag_in = dram.tile([NPCP, F], bf16, tag="yag")
            v = vp.tile([128, NB, F], fp32, tag="v")
            # transposes first, then matmuls: batching per engine avoids the
            # per-block PE<->Scalar convoy (xT holds all 49 blocks)
            for t in range(NB):
                xT_ps = ps.tile([128, 128], fp32, tag="xt_ps")
                nc.tensor.transpose(xT_ps[:], x0[:, t, :], ident[:])
                nc.scalar.activation(xT[:, t, :], xT_ps[:], Act.Copy)
            for t in range(NB):
                mm_block(t, xT, Ws[0], cbbcs[0], v, y_ag_in)

            for lt in range(NL * repeats):
                l = lt % NL
                last = lt == NL * repeats - 1
                nl = (lt + 1) % NL

                y_all = dram.tile([NTAB, F], bf16, tag="yall", addr_space="Shared")
                nc.gpsimd.collective_compute(
                    "AllGather",
                    Alu.bypass,
                    replica_groups=[list(range(NCORES))],
                    ins=[y_ag_in.opt()],
                    outs=[y_all.opt()],
                )

                # ---- gather + segment-max (+ next-layer matmul) phase ----
                if not last:
                    xT = xTp.tile([128, NB, 128], fp32, tag="xT")
                    y_ag_in = dram.tile([NPCP, F], bf16, tag="yag")
                    vn = vp.tile([128, NB, F], fp32, tag="v")
                for (b0, nbl, aoff, acnt, boff, bcnt) in groups:
                    gA = gap.tile([128, CAP, F], bf16, tag="ga")
                    gB = gbp.tile([128, CAP, F], bf16, tag="gb")
                    # Q7 gather ucode scratch caps num_idxs at 1024 (8 chunks)
                    for o in range(0, acnt, 8):
                        n = min(8, acnt - o)
                        nc.gpsimd.dma_gather(
                            gA[:, o : o + n, :], y_all[:, :],
                            idxA[:, (aoff + o) * 8 : (aoff + o + n) * 8],
                            n * 128, n * 128, F,
                            queue_num=qctr[0] % NQ,
                        )
                        qctr[0] += 1
                    for o in range(0, bcnt, 8):
                        n = min(8, bcnt - o)
                        nc.gpsimd.dma_gather(
                            gB[:, o : o + n, :], y_all[BASE_B:, :],
                            idxB[:, (boff + o) * 8 : (boff + o + n) * 8],
                            n * 128, n * 128, F,
                            queue_num=qctr[0] % NQ,
                        )
                        qctr[0] += 1
                    ka = 0
                    kb = 0
                    wmax = window_max
                    for b in range(b0, b0 + nbl):
                        ha, hb = int(KA[b]), int(KB[b])
                        tS = tp.tile([128, F], fp32, tag="ts")
                        if ha > 0 and hb > 0:
                            rA = wmax(gA, ka, ha, "ta")
                            rB = wmax(gB, kb, hb, "tb")
                            tM = tp.tile([128, F], fp32, tag="tm")
                            nc.vector.tensor_tensor(out=tM[:], in0=rA, in1=rB,
                                                    op=Alu.max)
                            nc.vector.tensor_tensor(out=tS[:], in0=tM[:],
                                                    in1=v[:, b, :], op=Alu.add)
                        elif ha > 0 or hb > 0:
                            rA = (wmax(gA, ka, ha, "ta") if ha > 0
                                  else wmax(gB, kb, hb, "tb"))
                            nc.vector.tensor_tensor(out=tS[:], in0=rA,
                                                    in1=v[:, b, :], op=Alu.add)
                        else:
                            nc.vector.memset(tS[:], NEG)
                        if last:
                            xn = tp.tile([128, F], fp32, tag="xn")
                            nc.scalar.activation(xn[:], tS[:], Act.Relu)
                            nc.sync.dma_start(xout_r[:, b, :], xn[:])
                        else:
                            xT_ps = ps.tile([128, 128], fp32, tag="xt_ps")
                            nc.tensor.transpose(xT_ps[:], tS[:], ident[:])
                            nc.scalar.activation(xT[:, b, :], xT_ps[:], Act.Relu)
                            mm_block(b, xT, Ws[nl], cbbcs[nl], vn, y_ag_in)
                        ka += ha
                        kb += hb
                if not last:
                    v = vn

    nc.compile()
    return nc


# ----------------------------------------------------------------------------
# numpy emulation of the device dataflow (for validating prep structures)
# ----------------------------------------------------------------------------

def _bf16(x):
    u = x.astype(np.float32).view(np.uint32)
    u = (u + 0x8000 + ((u >> 16) & 1)) & 0xFFFF0000
    return u.view(np.float32)


def _emulate(g, feats_dev, wcat, cb):
    KA, KB = g["KA"], g["KB"]
    x = feats_dev.copy()  # [NCORES, NPCP, F] sigma-ordered
    for l in range(NL):
        y_sh = _bf16(np.einsum("cnf,fk->cnk", x, wcat[l, :, :F]))
        v = np.einsum("cnf,fk->cnk", x, wcat[l, :, F:]) + cb[l]
        y_sh[:, :N_PHANTOM, :] = NEG
        table = y_sh.reshape(NTAB, F)
        xn = np.empty_like(x)
        for c in range(NCORES):
            gA = table[g["idxA_flat"][c].astype(np.int64)]
            gB = table[BASE_B + g["idxB_flat"][c].astype(np.int64)]
            gA = gA.reshape(g["CA"], 128, F)
            gB = gB.reshape(g["CB"], 128, F)
            for b in range(NB):
                a0, b0 = g["cbA"][b], g["cbB"][b]
                parts = []
                if KA[b] > 0:
                    parts.append(gA[a0 : a0 + KA[b]].max(0))
                if KB[b] > 0:
                    parts.append(gB[b0 : b0 + KB[b]].max(0))
                agg = np.full((128, F), NEG, np.float32) if not parts else (
                    parts[0] if len(parts) == 1 else np.maximum(*parts))
                xn[c, b * 128 : (b + 1) * 128] = np.maximum(
                    agg + v[c, b * 128 : (b + 1) * 128], 0.0)
        x = xn
    return x


def _make_in_maps(g, feats_dev, wcat, cb):
    in_maps = []
    for c in range(NCORES):
        in_maps.append({
            "xin": np.ascontiguousarray(feats_dev[c]),
            "idxA": np.ascontiguousarray(g["idxA"][c]),
            "idxB": np.ascontiguousarray(g["idxB"][c]),
            "wcat": wcat,
            "cb": cb,
        })
    return in_maps


def _feats_dev(g, feats):
    feats = np.asarray(feats, np.float32)
    fd = np.zeros((NCORES, NPCP, F), np.float32)
    core = np.arange(N) // NPC
    fd[core, g["pos"]] = feats
    return fd


def _assemble(g, results):
    out_sh = np.stack([r["xout"] for r in results])  # [NCORES, NPCP, F]
    core = np.arange(N) // NPC
    return np.ascontiguousarray(out_sh[core, g["pos"]])


def run(feats, src, dst, theta_w, theta_b, phi_w, phi_b, trace=False, repeats=1):
    from concourse.bass_utils import run_bass_kernel_spmd

    src = np.asarray(src)
    dst = np.asarray(dst)
    key = (src.tobytes()[:64], dst.tobytes()[:64], len(src))
    if _cache.get("graph_key") != key:
        _cache.clear()
        _cache["graph"] = _prep_graph(src, dst)
        _cache["graph_key"] = key
    g = _cache["graph"]
    nck = ("nc", repeats)
    if nck not in _cache:
        _cache[nck] = _build_kernel(g, repeats=repeats)
    nc = _cache[nck]

    wcat, cb = _prep_weights(theta_w, theta_b, phi_w, phi_b)
    feats_dev = _feats_dev(g, feats)
    in_maps = _make_in_maps(g, feats_dev, wcat, cb)
    res = run_bass_kernel_spmd(nc, in_maps, core_ids=list(range(NCORES)),
                               trace=trace)
    out = _assemble(g, res.results)
    return out, res


def kernel(feats, src, dst, theta_w, theta_b, phi_w, phi_b):
    out, _ = run(feats, src, dst, theta_w, theta_b, phi_w, phi_b)
    return out
